# revision 8
# baseline (speedup 1.0000x reference)
"""Masked dot-product attention (ESIM masked_softmax) Trainium2 Bass kernel.

Math (per batch):
    s   = q @ k^T ; t = s * m  (== q @ (k*m)^T, exact since m is 0/1)
    p   = exp(t) * m / sum_k(exp(t) * m)   (max-subtraction cancels; |s|<~50
                                            so exp() stays in fp32 range)
    out = p @ v = (exp(t) @ [v*m | m]) -> numerator | denominator

Device mapping (per core, 2 batches, data-parallel over 8 cores):
  - masked key rows are compacted away on the host (kept rows first, zero-mask
    padding to LKC=1792), shrinking every O(Lq*Lk) stage by ~12%.
  - the host pre-builds ONE PE-ready packed buffer per batch so the device
    does ZERO layout prep and the head DMA is a single descriptor/partition:
      qkv [128, 3854] f16 = [ kmt 896 | qt 2048 | vme 910 (bf16 bits) ]
      kmt [128, 7, 128] f16 : partition = (pair-half, d); kmt[0:64, j] is
                              (k*m)^T of key block 2j, kmt[64:128, j] of 2j+1.
      qt  [128, 2048] f16   : q^T duplicated into both partition halves so
                              both row-tiles of the PE stream their own rhs.
      vme [128, 14, 65] bf16: [v*m | m] packed per key block (partition =
                              key-in-block); column 64 of the PV accumulation
                              is the softmax denominator for free.
    split as crit = [kmt | qt[:512]] (first S matmul unblocks) + rest.
  - a few warmup matmuls on a memset scratch run during the head DMA so the
    PE HAM clock-gate is released before the real stream starts.
  - scores are computed TRANSPOSED (k on partitions, q free) in a SINGLE fp16
    pass (row-paired K=64 matmuls; fp16 accuracy is inside the 2e-2 budget)
    so exp(s^T) is directly the rhs of the PV matmul.
  - exp is split between ACT (exact table lookup, bf16 out) and DVE
    (Schraudolph 2^x bit trick: one fused tensor_scalar mult+add with int16
    output whose bits ARE the bf16 weight). DVE takes ND of 14 tiles/half.
  - PV accumulates with stationary [v*m | m] bf16; weights lag PV_LAG j-groups
    behind their exps so the in-order PE never stalls on ACT/DVE.
  - out^T [65, Lq] (numerator rows 0..63, denominator row 64) is stored
    transposed; the host does the divide + final transpose.
"""

import os
import sys

import numpy as np

sys.path.insert(0, "/opt/trn_rl_repo")

import concourse.bacc as bacc
import concourse.bass as bass
import concourse.mybir as mybir
import concourse.tile as tile
from concourse import bass_utils

B, LQ, LK, D = 16, 2048, 2048, 64
NCORES = 8
PB = B // NCORES  # batches per core
P = 128
LKC = 1792  # compacted key length (14 blocks); fallback to 2048 if exceeded

ND = int(os.environ.get("ATT_ND", "6"))  # exp tiles per half on DVE (of 14)
SCHRAUD_C16 = float(os.environ.get("ATT_C16", "5.6"))
PV_LAG = int(os.environ.get("ATT_LAG", "2"))
WARM = int(os.environ.get("ATT_WARM", "6"))  # warmup matmuls during head DMA
S_ORDER = os.environ.get("ATT_SORDER", "AABB")  # AABB | ABAB
EXPSPLIT = os.environ.get("ATT_EXPSPLIT", "1") == "1"  # exp(sA) between sides
PRELDW = os.environ.get("ATT_PRELDW", "0") == "1"  # pre-load PV stationary

F32 = mybir.dt.float32
F16 = mybir.dt.float16
BF16 = mybir.dt.bfloat16
I16 = mybir.dt.int16
EXP = mybir.ActivationFunctionType.Exp
MULT = mybir.AluOpType.mult
ADD = mybir.AluOpType.add

# exp(x) ~= bitcast_bf16(int16(x * 2^7/ln2 + (127*2^7 - C16))): the int16
# affine builds the bf16 bit pattern of 2^(x*log2 e) directly (Schraudolph).
SCH_A = float(128.0 / np.log(2.0))
SCH_B = float(127.0 * 128.0 - SCHRAUD_C16)


def _attention_core(tc, crit_d, rest_d, o_d, nkb):
    nc = tc.nc
    npair = nkb // 2
    kc = npair * P          # kmt cols
    vc = nkb * (D + 1)      # vme cols
    tot = kc + LQ + vc
    crit_cols = kc + 512
    pools = []

    def pool(name, bufs, space="SBUF"):
        p = tc.alloc_tile_pool(name=name, bufs=bufs, space=space)
        pools.append(p)
        return p

    warmp = pool("warm", 1)
    stage = pool("stage", 2)
    wtp = pool("wt", 8)
    outp = pool("outp", 2)
    ps_s = pool("ps_s", 3, space="PSUM")  # 3 x [128,1024] = 6 banks
    ps_pv = pool("ps_pv", 2, space="PSUM")  # 2 x [65,512] = 2 banks

    # warmup: release the HAM clock-gate while the head DMA is in flight
    if WARM:
        wsrc = warmp.tile([P, 512], F16, tag="wsrc")
        nc.gpsimd.memset(wsrc, 0)
        wps = ps_s.tile([P, 1024], F32, tag="s", name="warm_ps")
        for _ in range(WARM):
            nc.tensor.matmul(
                wps[:, 0:512], wsrc[:, 0:128], wsrc, start=True, stop=True
            )

    # which (j, side) exp tiles go to DVE per half. B-side only: the 3-slot
    # PSUM rotation makes sB(j) wait on exp(sA(j-1)) (1-j slack) while sA(j)
    # waits on exp(sB(j-2)) (2-j slack) — so keep every sA exp on the faster
    # ACT path and give the slow DVE tiles the 2-j slack B side.
    dve_tiles = {(j, 1) for j in range(min(ND, npair))}
    if ND > npair:
        dve_tiles |= {(j, 0) for j in range(ND - npair)}

    def prep_io(b, head=False):
        qkv = stage.tile([P, tot], F16, tag="qkv", name=f"qkv{b}")
        crit_eng = nc.sync if head else nc.scalar
        rest_eng = nc.gpsimd
        crit_eng.dma_start(out=qkv[:, 0:crit_cols], in_=crit_d[b])
        rest_eng.dma_start(out=qkv[:, crit_cols:tot], in_=rest_d[b])
        bc = {}
        bc["kmt"] = qkv[:, 0:kc].rearrange("p (j x) -> p j x", j=npair)
        bc["qt"] = qkv[:, kc : kc + LQ]
        bc["vme"] = (
            qkv[:, kc + LQ : tot].bitcast(BF16).rearrange("p (t e) -> p t e", t=nkb)
        )
        return bc

    def main_half(b, bc, h):
        kmt, qt, vme = bc["kmt"], bc["qt"], bc["vme"]
        pvc = [
            ps_pv.tile([D + 1, 512], F32, tag="pv", name=f"pv{b}_{h}_{c}")
            for c in range(2)
        ]

        def emit_pv(j, wA, wB):
            # c innermost: consecutive matmuls alternate PSUM banks, so the
            # accumulate never waits on its own bank's drain.
            if PRELDW:
                nc.tensor.ldweights(vme[:, 2 * j, :])
            for kb, w in ((2 * j, wA), (2 * j + 1, wB)):
                for c in range(2):
                    cs = slice(c * 512, (c + 1) * 512)
                    nc.tensor.matmul(
                        pvc[c], vme[:, kb, :], w[:, cs],
                        start=(kb == 0), stop=(kb == nkb - 1),
                    )

        pend = []
        for j in range(npair):
            sA = ps_s.tile([P, 1024], F32, tag="s", name=f"sA{b}_{h}_{j}")
            sB = ps_s.tile([P, 1024], F32, tag="s", name=f"sB{b}_{h}_{j}")

            def s_mm(side, c):
                s_t = sA if side == 0 else sB
                rows = slice(0, 64) if side == 0 else slice(64, 128)
                qs = slice(h * 1024 + c * 512, h * 1024 + (c + 1) * 512)
                cs = slice(c * 512, (c + 1) * 512)
                nc.tensor.matmul(
                    s_t[:, cs], kmt[rows, j, :], qt[rows, qs],
                    start=True, stop=True, tile_position=(0 if side == 0 else 64, 0),
                )

            wA = wtp.tile([P, 1024], BF16, tag="wt", name=f"wA{b}_{h}_{j}")
            wB = wtp.tile([P, 1024], BF16, tag="wt", name=f"wB{b}_{h}_{j}")

            def emit_exp(side):
                s_t, w_t = (sA, wA) if side == 0 else (sB, wB)
                if (j, side) in dve_tiles:
                    nc.vector.tensor_scalar(
                        out=w_t.bitcast(I16), in0=s_t,
                        scalar1=SCH_A, scalar2=SCH_B, op0=MULT, op1=ADD,
                    )
                else:
                    nc.scalar.activation(out=w_t, in_=s_t, func=EXP)

            if S_ORDER == "AABB":
                # share one LDWEIGHTS per side across both c chunks; launch
                # each side's exp as soon as that side's scores are complete
                for side in (0, 1):
                    for c in (0, 1):
                        s_mm(side, c)
                    if EXPSPLIT:
                        emit_exp(side)
                if not EXPSPLIT:
                    emit_exp(0)
                    emit_exp(1)
            else:
                for c in (0, 1):
                    for side in (0, 1):
                        s_mm(side, c)
                emit_exp(0)
                emit_exp(1)
            pend.append((j, wA, wB))
            if len(pend) > PV_LAG:
                emit_pv(*pend.pop(0))
        while pend:
            emit_pv(*pend.pop(0))

        # drain accumulators to SBUF (frees the pv psum slots) and store
        outT = outp.tile([D + 1, 1024], F32, tag="outT", name=f"outT{b}_{h}")
        for c in range(2):
            nc.vector.tensor_copy(outT[:, c * 512 : (c + 1) * 512], pvc[c])
        nc.sync.dma_start(out=o_d[b][:, h * 1024 : (h + 1) * 1024], in_=outT)

    bcs = [prep_io(0, head=True)]
    if PB > 1:
        bcs.append(prep_io(1))
    for b in range(PB):
        for h in range(2):
            main_half(b, bcs[b], h)

    for p in reversed(pools):
        p.release()


_NC_CACHE = {}


def _build_nc(nkb):
    if nkb in _NC_CACHE:
        return _NC_CACHE[nkb]
    npair = nkb // 2
    kc = npair * P
    vc = nkb * (D + 1)
    nc = bacc.Bacc(None, target_bir_lowering=False, debug=False)
    crit_d = nc.dram_tensor("crit", [PB, P, kc + 512], F16, kind="ExternalInput")
    rest_d = nc.dram_tensor(
        "rest", [PB, P, LQ - 512 + vc], F16, kind="ExternalInput"
    )
    o_d = nc.dram_tensor("out", [PB, D + 1, LQ], F32, kind="ExternalOutput")
    with tile.TileContext(nc) as tc:
        _attention_core(tc, crit_d, rest_d, o_d, nkb)
    nc.compile()
    _NC_CACHE[nkb] = nc
    return nc


def kernel(q, k, v, v_mask, _trace=False, _tmpdir=None):
    import ml_dtypes

    q = np.ascontiguousarray(q, dtype=np.float32)
    k = np.ascontiguousarray(k, dtype=np.float32)
    v = np.ascontiguousarray(v, dtype=np.float32)
    v_mask = np.ascontiguousarray(v_mask, dtype=np.float32)
    assert q.shape == (B, LQ, D), q.shape

    # fold the 0/1 mask into k and v on the host (exact; masked key rows
    # contribute exp(0)*0 = 0 to both numerator and denominator)
    k = k * v_mask[:, :, None]
    v = v * v_mask[:, :, None]
    counts = (v_mask > 0.5).sum(axis=1)
    if counts.max() <= LKC:
        # kept key rows first (stable), zero-mask padding after
        order = np.argsort(v_mask <= 0.5, axis=1, kind="stable")[:, :LKC]
        kk = np.take_along_axis(k, order[:, :, None], axis=1)
        vv = np.take_along_axis(v, order[:, :, None], axis=1)
        mm = np.take_along_axis(v_mask, order, axis=1)
        nkb = LKC // P
    else:
        kk, vv, mm = k, v, v_mask
        nkb = LK // P
    npair = nkb // 2

    # PE-ready host layouts (see module docstring)
    kmt = np.ascontiguousarray(
        kk.reshape(B, npair, 2, P, D).transpose(0, 2, 4, 1, 3).reshape(B, P, npair * P)
    ).astype(np.float16)
    qT = np.ascontiguousarray(q.transpose(0, 2, 1))
    qt = np.concatenate([qT, qT], axis=1).astype(np.float16)  # [B, 128, LQ]
    vme = np.concatenate([vv, mm[:, :, None]], axis=2)  # [B, LKC, 65]
    vme = np.ascontiguousarray(
        vme.reshape(B, nkb, P, D + 1).transpose(0, 2, 1, 3).reshape(B, P, nkb * (D + 1))
    ).astype(ml_dtypes.bfloat16)
    vme16 = vme.view(np.float16)  # raw bf16 bits carried in the f16 buffer

    crit = np.ascontiguousarray(np.concatenate([kmt, qt[:, :, 0:512]], axis=2))
    rest = np.ascontiguousarray(np.concatenate([qt[:, :, 512:LQ], vme16], axis=2))

    nc = _build_nc(nkb)
    in_maps = [
        {
            "crit": np.ascontiguousarray(crit[i * PB : (i + 1) * PB]),
            "rest": np.ascontiguousarray(rest[i * PB : (i + 1) * PB]),
        }
        for i in range(NCORES)
    ]
    res = bass_utils.run_bass_kernel_spmd(
        nc, in_maps, core_ids=list(range(NCORES)), trace=_trace, tmpdir=_tmpdir
    )
    oT = np.concatenate([r["out"] for r in res.results], axis=0)  # [B, 65, LQ]
    out = np.ascontiguousarray(
        (oT[:, 0:D, :] / oT[:, D : D + 1, :]).transpose(0, 2, 1)
    ).astype(np.float32)
    if _trace:
        kernel.last_results = res
    return out


# revision 17
# speedup vs baseline: 1.2211x; 1.2211x over previous
"""Masked dot-product attention (ESIM masked_softmax) Trainium2 Bass kernel.

Math (per batch):
    s   = q @ k^T ; t = s * m  (== q @ (k*m)^T, exact since m is 0/1)
    p   = exp(t) * m / sum_k(exp(t) * m)   (max-subtraction cancels; |s|<~50
                                            so exp() stays in fp32 range)
    out = p @ v = (exp(t) @ [v*m | m]) -> numerator | denominator

Device mapping (per core, 2 batches, data-parallel over 8 cores):
  - masked key rows are compacted away on the host (kept rows first, zero-mask
    padding to LKC=1792), shrinking every O(Lq*Lk) stage by ~12%.
  - the host pre-builds ONE PE-ready packed buffer per batch so the device
    does ZERO layout prep and every DMA is one descriptor per partition:
      qkv [128, 3854] f16 = [ kmt_j0 | qt_lo | kmt_j1..6 | qt_hi | vme ]
      kmt [.., 7, 128] f16  : partition = (pair-half, d); kmt j slice is
                              (k*m)^T of key blocks 2j / 2j+1 on the two
                              partition halves.
      qt  [128, 2048] f16   : q^T duplicated into both partition halves so
                              both row-tiles of the PE stream their own rhs.
      vme [128, 14, 65] bf16: [v*m | m] packed per key block (partition =
                              key-in-block); column 64 of the PV accumulation
                              is the softmax denominator for free.
    DMA'd in 3 sequential pieces on one ring (head = kmt_j0 + first q half
    unblocks the j-loop early; the rest streams in behind it).
  - a few warmup matmuls on a memset scratch run during the head DMA so the
    PE HAM clock-gate is released before the real stream starts.
  - scores are computed TRANSPOSED (k on partitions, q free) in a SINGLE fp16
    pass (row-paired K=64 matmuls; fp16 accuracy is inside the 2e-2 budget)
    so exp(s^T) is directly the rhs of the PV matmul.
  - exp is split between ACT (exact table lookup, bf16 out) and DVE
    (Schraudolph 2^x bit trick: one fused tensor_scalar mult+add with int16
    output whose bits ARE the bf16 weight). DVE takes ND of 14 tiles/half.
  - PV accumulates with stationary [v*m | m] bf16; weights lag PV_LAG j-groups
    behind their exps so the in-order PE never stalls on ACT/DVE.
  - out^T [65, Lq] (numerator rows 0..63, denominator row 64) is stored
    transposed; the host does the divide + final transpose.
"""

import os
import sys

import numpy as np

sys.path.insert(0, "/opt/trn_rl_repo")

import concourse.bacc as bacc
import concourse.bass as bass
import concourse.mybir as mybir
import concourse.tile as tile
from concourse import bass_utils

B, LQ, LK, D = 16, 2048, 2048, 64
NCORES = 8
PB = B // NCORES  # batches per core
P = 128
LKC = 1792  # compacted key length (14 blocks); fallback to 2048 if exceeded

ND = int(os.environ.get("ATT_ND", "6"))  # exp tiles per half on DVE (of 14)
SCHRAUD_C16 = float(os.environ.get("ATT_C16", "5.6"))
PV_LAG = int(os.environ.get("ATT_LAG", "2"))
WARM = int(os.environ.get("ATT_WARM", "3"))  # warmup matmuls during head DMA
S_ORDER = os.environ.get("ATT_SORDER", "AABB")  # AABB | ABAB
EXPSPLIT = os.environ.get("ATT_EXPSPLIT", "1") == "1"  # exp(sA) between sides


F32 = mybir.dt.float32
F16 = mybir.dt.float16
BF16 = mybir.dt.bfloat16
I16 = mybir.dt.int16
EXP = mybir.ActivationFunctionType.Exp
MULT = mybir.AluOpType.mult
ADD = mybir.AluOpType.add

# exp(x) ~= bitcast_bf16(int16(x * 2^7/ln2 + (127*2^7 - C16))): the int16
# affine builds the bf16 bit pattern of 2^(x*log2 e) directly (Schraudolph).
SCH_A = float(128.0 / np.log(2.0))
SCH_B = float(127.0 * 128.0 - SCHRAUD_C16)


def _attention_core(tc, qkv_d, o_d, nkb):
    nc = tc.nc
    npair = nkb // 2
    kc = npair * P          # kmt cols
    vc = nkb * (D + 1)      # vme cols
    tot = kc + LQ + vc
    c0 = P + 1024           # head region: kmt j=0 + qt half 0
    c1 = kc + LQ            # end of mid region
    pools = []

    def pool(name, bufs, space="SBUF"):
        p = tc.alloc_tile_pool(name=name, bufs=bufs, space=space)
        pools.append(p)
        return p

    warmp = pool("warm", 1)
    stage = pool("stage", 2)
    wtp = pool("wt", 12)
    outp = pool("outp", 2)
    ps_s = pool("ps_s", 3, space="PSUM")  # 3 x [128,1024] = 6 banks
    ps_pv = pool("ps_pv", 2, space="PSUM")  # 2 x [65,512] = 2 banks

    # warmup: release the HAM clock-gate while the head DMA is in flight
    if WARM:
        wsrc = warmp.tile([P, 512], F16, tag="wsrc")
        nc.gpsimd.memset(wsrc, 0)
        wps = ps_s.tile([P, 1024], F32, tag="s", name="warm_ps")
        for _ in range(WARM):
            nc.tensor.matmul(
                wps[:, 0:512], wsrc[:, 0:128], wsrc, start=True, stop=True
            )

    # which (j, side) exp tiles go to DVE per half. B-side only: the 3-slot
    # PSUM rotation makes sB(j) wait on exp(sA(j-1)) (1-j slack) while sA(j)
    # waits on exp(sB(j-2)) (2-j slack) — so keep every sA exp on the faster
    # ACT path and give the slow DVE tiles the 2-j slack B side.
    dve_tiles = {(j, 1) for j in range(1, min(ND, npair - 1) + 1)}
    if ND > npair - 1:
        dve_tiles |= {(0, 1)}
    if ND > npair:
        dve_tiles |= {(j, 0) for j in range(ND - npair)}

    def prep_io(b, head=False):
        # all input DMAs sequential on ONE ring: the head-critical region
        # (kmt j=0 + the first q half) lands first so the j-loop starts while
        # the rest streams in behind it
        qkv = stage.tile([P, tot], F16, tag="qkv", name=f"qkv{b}")
        if head:
            nc.sync.dma_start(out=qkv[:, 0:c0], in_=qkv_d[b][:, 0:c0])
            nc.sync.dma_start(out=qkv[:, c0:c1], in_=qkv_d[b][:, c0:c1])
        else:
            nc.sync.dma_start(out=qkv[:, 0:c1], in_=qkv_d[b][:, 0:c1])
        nc.sync.dma_start(out=qkv[:, c1:tot], in_=qkv_d[b][:, c1:tot])
        bc = {}
        bc["kmt0"] = qkv[:, 0:P]
        bc["kmtR"] = qkv[:, c0 : c0 + (npair - 1) * P].rearrange(
            "p (j x) -> p j x", j=npair - 1
        )
        bc["qt_lo"] = qkv[:, P:c0]
        bc["qt_hi"] = qkv[:, c0 + (npair - 1) * P : c1]
        bc["vme"] = (
            qkv[:, c1:tot].bitcast(BF16).rearrange("p (t e) -> p t e", t=nkb)
        )
        return bc

    pend = []  # PV groups carried across halves: PE keeps accumulating the
    # previous half's PV while the new half's S groups fill the exp pipeline

    def flush_one():
        e = pend.pop(0)
        e["fn"](*e["args"])
        if e.get("fin"):
            e["fin"]()

    def main_half(b, bc, h, last=False):
        vme = bc["vme"]
        qt = bc["qt_lo"] if h == 0 else bc["qt_hi"]

        def kmt_ap(rows, j):
            if j == 0:
                return bc["kmt0"][rows, :]
            return bc["kmtR"][rows, j - 1, :]
        pvc = [
            ps_pv.tile([D + 1, 512], F32, tag="pv", name=f"pv{b}_{h}_{c}")
            for c in range(2)
        ]

        def emit_pv(j, wA, wB):
            # c innermost: consecutive matmuls alternate PSUM banks, so the
            # accumulate never waits on its own bank's drain.
            for kb, w in ((2 * j, wA), (2 * j + 1, wB)):
                for c in range(2):
                    cs = slice(c * 512, (c + 1) * 512)
                    nc.tensor.matmul(
                        pvc[c], vme[:, kb, :], w[:, cs],
                        start=(kb == 0), stop=(kb == nkb - 1),
                    )

        for j in range(npair):
            sA = ps_s.tile([P, 1024], F32, tag="s", name=f"sA{b}_{h}_{j}")
            sB = ps_s.tile([P, 1024], F32, tag="s", name=f"sB{b}_{h}_{j}")

            def s_mm(side, c):
                s_t = sA if side == 0 else sB
                rows = slice(0, 64) if side == 0 else slice(64, 128)
                qs = slice(c * 512, (c + 1) * 512)
                cs = slice(c * 512, (c + 1) * 512)
                nc.tensor.matmul(
                    s_t[:, cs], kmt_ap(rows, j), qt[rows, qs],
                    start=True, stop=True, tile_position=(0 if side == 0 else 64, 0),
                )

            wA = wtp.tile([P, 1024], BF16, tag="wt", name=f"wA{b}_{h}_{j}")
            wB = wtp.tile([P, 1024], BF16, tag="wt", name=f"wB{b}_{h}_{j}")

            def emit_exp(side):
                s_t, w_t = (sA, wA) if side == 0 else (sB, wB)
                if (j, side) in dve_tiles:
                    nc.vector.tensor_scalar(
                        out=w_t.bitcast(I16), in0=s_t,
                        scalar1=SCH_A, scalar2=SCH_B, op0=MULT, op1=ADD,
                    )
                else:
                    nc.scalar.activation(out=w_t, in_=s_t, func=EXP)

            if S_ORDER == "AABB":
                # share one LDWEIGHTS per side across both c chunks; launch
                # each side's exp as soon as that side's scores are complete
                for side in (0, 1):
                    for c in (0, 1):
                        s_mm(side, c)
                    if EXPSPLIT:
                        emit_exp(side)
                if not EXPSPLIT:
                    emit_exp(0)
                    emit_exp(1)
            else:
                for c in (0, 1):
                    for side in (0, 1):
                        s_mm(side, c)
                emit_exp(0)
                emit_exp(1)
            pend.append({"fn": emit_pv, "args": (j, wA, wB)})
            if len(pend) > PV_LAG:
                flush_one()

        def finisher():
            # drain accumulators (frees the pv psum slots) and store; direct
            # PSUM->DRAM skips the SBUF bounce, else copy then store per chunk
            outT = outp.tile([D + 1, 1024], F32, tag="outT", name=f"outT{b}_{h}")
            for c in range(2):
                dst = o_d[b][:, h * 1024 + c * 512 : h * 1024 + (c + 1) * 512]
                eng = nc.sync if c == 0 else nc.gpsimd
                cs = slice(c * 512, (c + 1) * 512)
                nc.vector.tensor_copy(outT[:, cs], pvc[c])
                eng.dma_start(out=dst, in_=outT[:, cs])

        pend[-1]["fin"] = finisher
        if last:
            while pend:
                flush_one()

    bcs = [prep_io(0, head=True)]
    if PB > 1:
        bcs.append(prep_io(1))
    for b in range(PB):
        for h in range(2):
            main_half(b, bcs[b], h, last=(b == PB - 1 and h == 1))

    for p in reversed(pools):
        p.release()


_NC_CACHE = {}


def _build_nc(nkb):
    if nkb in _NC_CACHE:
        return _NC_CACHE[nkb]
    npair = nkb // 2
    kc = npair * P
    vc = nkb * (D + 1)
    nc = bacc.Bacc(None, target_bir_lowering=False, debug=False)
    qkv_d = nc.dram_tensor(
        "qkv", [PB, P, kc + LQ + nkb * (D + 1)], F16, kind="ExternalInput"
    )
    o_d = nc.dram_tensor("out", [PB, D + 1, LQ], F32, kind="ExternalOutput")
    with tile.TileContext(nc) as tc:
        _attention_core(tc, qkv_d, o_d, nkb)
    nc.compile()
    _NC_CACHE[nkb] = nc
    return nc


def kernel(q, k, v, v_mask, _trace=False, _tmpdir=None):
    import ml_dtypes

    q = np.ascontiguousarray(q, dtype=np.float32)
    k = np.ascontiguousarray(k, dtype=np.float32)
    v = np.ascontiguousarray(v, dtype=np.float32)
    v_mask = np.ascontiguousarray(v_mask, dtype=np.float32)
    assert q.shape == (B, LQ, D), q.shape

    # fold the 0/1 mask into k and v on the host (exact; masked key rows
    # contribute exp(0)*0 = 0 to both numerator and denominator)
    k = k * v_mask[:, :, None]
    v = v * v_mask[:, :, None]
    counts = (v_mask > 0.5).sum(axis=1)
    if counts.max() <= LKC:
        # kept key rows first (stable), zero-mask padding after
        order = np.argsort(v_mask <= 0.5, axis=1, kind="stable")[:, :LKC]
        kk = np.take_along_axis(k, order[:, :, None], axis=1)
        vv = np.take_along_axis(v, order[:, :, None], axis=1)
        mm = np.take_along_axis(v_mask, order, axis=1)
        nkb = LKC // P
    else:
        kk, vv, mm = k, v, v_mask
        nkb = LK // P
    npair = nkb // 2

    # PE-ready host layouts (see module docstring)
    kmt = np.ascontiguousarray(
        kk.reshape(B, npair, 2, P, D).transpose(0, 2, 4, 1, 3).reshape(B, P, npair * P)
    ).astype(np.float16)
    qT = np.ascontiguousarray(q.transpose(0, 2, 1))
    qt = np.concatenate([qT, qT], axis=1).astype(np.float16)  # [B, 128, LQ]
    vme = np.concatenate([vv, mm[:, :, None]], axis=2)  # [B, LKC, 65]
    vme = np.ascontiguousarray(
        vme.reshape(B, nkb, P, D + 1).transpose(0, 2, 1, 3).reshape(B, P, nkb * (D + 1))
    ).astype(ml_dtypes.bfloat16)
    vme16 = vme.view(np.float16)  # raw bf16 bits carried in the f16 buffer

    qkv_h = np.ascontiguousarray(
        np.concatenate(
            [
                kmt[:, :, 0:P],
                qt[:, :, 0:1024],
                kmt[:, :, P:],
                qt[:, :, 1024:LQ],
                vme16,
            ],
            axis=2,
        )
    )

    nc = _build_nc(nkb)
    in_maps = [
        {"qkv": np.ascontiguousarray(qkv_h[i * PB : (i + 1) * PB])}
        for i in range(NCORES)
    ]
    res = bass_utils.run_bass_kernel_spmd(
        nc, in_maps, core_ids=list(range(NCORES)), trace=_trace, tmpdir=_tmpdir
    )
    oT = np.concatenate([r["out"] for r in res.results], axis=0)  # [B, 65, LQ]
    out = np.ascontiguousarray(
        (oT[:, 0:D, :] / oT[:, D : D + 1, :]).transpose(0, 2, 1)
    ).astype(np.float32)
    if _trace:
        kernel.last_results = res
    return out
